# revision 6
# baseline (speedup 1.0000x reference)
"""Trainium2 Bass kernel for nn_Cube_Norm (segment min/max normalize).

Reference semantics (per graph g of 256 nodes, per dim d):
    tmax = max_n x[g,n,d]; tmin = min_n x[g,n,d]
    mid = (tmax+tmin)/2; ldv = max((tmax-tmin)/2, 1e-12)
    out[g,n,d] = (x[g,n,d] - mid) / ldv

Sharding: 1024 graphs -> 8 cores x 128 graphs (row-sharded at graph
boundaries). Per core, 4 rounds of 32 graphs; each graph occupies 4 SBUF
partitions (64 nodes each): every round is a [128, 19200] fp32 tile with
contiguous DMA in/out (exactly 2x HBM traffic), double-buffered.

Engine split. HW-measured: a DVE op whose two tensor operands are both
in SBUF locks the shared DVE/GpSimd SBUF port pair for the whole
instruction and fully serializes against any GpSimd op (probe: DVE TT
stretched 2.6us -> 15-20us, ending exactly when the GpSimd op ended).
A DVE TT with one operand in PSUM runs at full 1x rate fully overlapped
with GpSimd (probe: 1154ns for 960 elems at 100% GpSimd overlap) --
PSUM has its own DVE port and GpSimd cannot access PSUM. So:
  - Every DVE op here is mixed-operand (<=1 SBUF stream + PSUM stream):
    folds accumulate SBUF chunks into a PSUM ping-pong accumulator;
    the cross-partition stats tree alternates SBUF/PSUM outputs so each
    combine is (SBUF, PSUM); the normalize's mid/rinv broadcast operand
    reads a PSUM replica of the stats.
  - GpSimd concurrently runs the normalize majority slice (stock
    tensor_tensor, ~2.6-3.3 cyc/elem) straight in SBUF.
  - ScalarE (ACT, own ports, never contends) does all stats scaling:
    mid = 0.5*sum, rinv = exp(-ln(relu(0.5*diff - eps) + eps)) (the ACT
    Reciprocal is banned for accuracy; exp/ln error ~1e-4 scales
    multiplicatively with the output so rel-err stays ~1e-4), plus the
    SBUF->PSUM stats replica copy.
  - Loads ride the sync HWDGE ring; stores/stat DMAs ride the scalar
    ring, so stores never head-of-line-block loads.
The last round's normalize (no folds to overlap) rebalances toward DVE.
"""

import numpy as np

NUM_GRAPHS = 1024
NPG = 256            # nodes per graph
D = 300              # embed dim
N_CORES = 8
GPC = NUM_GRAPHS // N_CORES   # 128 graphs per core
ROWS_PER_CORE = GPC * NPG     # 32768
P = 128              # SBUF partitions
Q = 4                # partitions per graph
NPP = NPG // Q       # 64 nodes per partition
GPR = P // Q         # 32 graphs per round
ROUNDS = GPC // GPR  # 4
FREE = NPP * D       # 19200 fp32 per partition per round
ROWS_PER_ROUND = GPR * NPG    # 8192
EPS = 1e-12

# normalize node split: DVE gets [0:dn), GpSimd gets [dn:NPP) in two chunks.
DN = 19              # steady-state rounds (folds + tree share DVE)
DN_TAIL = 45         # last round (no folds -> DVE takes the majority)

CH = 1200            # fold chunk: 4 nodes x 300 dims; PSUM-sized
NCH = FREE // CH     # 16 chunks
PPW = 1536           # ping-pong slot width (3 PSUM banks each)

_CACHE = {}


def _split_multi_waits(nc, mybir, max_waits=1):
    """walrus in this container rejects >N sync waits on one instruction;
    hoist extras into standalone NOPs on the same engine just before."""
    n = 0
    for f in nc.m.functions:
        for bb in f.blocks:
            new_insts = []
            for inst in bb.instructions:
                si = inst.sync_info
                if si is not None and si.on_wait and len(si.on_wait) > max_waits:
                    extra = list(si.on_wait[: len(si.on_wait) - max_waits])
                    keep = list(si.on_wait[len(si.on_wait) - max_waits:])
                    for j, w in enumerate(extra):
                        new_insts.append(
                            mybir.InstNoOp(
                                name=f"{inst.name}-sw{j}",
                                sync_info=mybir.SyncInfo(on_wait=[w], on_update=[]),
                                bass_nofuse=True,
                                engine=inst.engine,
                            )
                        )
                        n += 1
                    inst.sync_info = mybir.SyncInfo(
                        on_wait=keep, on_update=list(si.on_update)
                    )
                new_insts.append(inst)
            bb.instructions.clear()
            for i in new_insts:
                bb.add_instruction(i)
    return n


def _build():
    import concourse.bass as bass
    import concourse.tile as tile
    from concourse import mybir

    F32 = mybir.dt.float32
    OP = mybir.AluOpType
    AF = mybir.ActivationFunctionType

    nc = bass.Bass()
    x = nc.dram_tensor("x", [ROWS_PER_CORE, D], F32, kind="ExternalInput")
    y = nc.dram_tensor("y", [ROWS_PER_CORE, D], F32, kind="ExternalOutput")

    # activation() lowers non-Copy float biases through the const-AP registry,
    # which only pre-registers 0.0/1.0 -- add the eps biases it will need.
    for v in (-EPS, EPS):
        cten = nc.alloc_sbuf_tensor(f"const-f32-eps{'-neg' if v < 0 else ''}",
                                    [128, 1], F32)
        nc.gpsimd.memset(cten.ap(), v)
        nc.const_aps.aps[(F32, v)] = cten.ap()
    nc.all_engine_barrier()

    with tile.TileContext(nc) as tc:
        with tc.tile_pool(name="data", bufs=2) as data_pool, \
             tc.tile_pool(name="st2", bufs=2) as st2_pool, \
             tc.tile_pool(name="st1", bufs=1) as st1_pool, \
             tc.psum_pool(name="ps", bufs=1) as ps_pool:
            live = {}  # r -> (t, rep_ps) awaiting normalize+store

            def emit_normalize(r_prev):
                """Normalize round r_prev in place and store; DVE (PSUM
                stats operand) takes nodes [0:dn), GpSimd the rest in two
                chunks (SBUF stats)."""
                tp, repp_ps, repp_sb = live.pop(r_prev)
                rowsp = slice(r_prev * ROWS_PER_ROUND, (r_prev + 1) * ROWS_PER_ROUND)
                tv3 = tp[:].rearrange("p (n d) -> p n d", n=NPP, d=D)
                yr = y[rowsp, :].rearrange("(p f) d -> p (f d)", p=P)
                dn = DN_TAIL if r_prev == ROUNDS - 1 else DN

                # GpSimd: two chunks, store each as soon as its mul lands
                gmid = (dn + NPP) // 2
                for lo, hi in ((dn, gmid), (gmid, NPP)):
                    h = hi - lo
                    mid_b = repp_sb[:, 0:D].unsqueeze(1).broadcast_to([P, h, D])
                    rinv_b = repp_sb[:, D:2 * D].unsqueeze(1).broadcast_to([P, h, D])
                    ns = slice(lo, hi)
                    nc.gpsimd.tensor_tensor(
                        tv3[:, ns, :], tv3[:, ns, :], mid_b, op=OP.subtract
                    )
                    nc.gpsimd.tensor_tensor(
                        tv3[:, ns, :], tv3[:, ns, :], rinv_b, op=OP.mult
                    )
                    nc.scalar.dma_start(
                        yr[:, lo * D:hi * D], tp[:, lo * D:hi * D]
                    )

                # DVE: nodes [0:dn) with the PSUM stats replica as operand
                mid_b = repp_ps[:, 0:D].unsqueeze(1).broadcast_to([P, dn, D])
                rinv_b = repp_ps[:, D:2 * D].unsqueeze(1).broadcast_to([P, dn, D])
                ns = slice(0, dn)
                nc.vector.tensor_tensor(
                    tv3[:, ns, :], tv3[:, ns, :], mid_b, op=OP.subtract
                )
                nc.vector.tensor_tensor(
                    tv3[:, ns, :], tv3[:, ns, :], rinv_b, op=OP.mult
                )
                nc.scalar.dma_start(yr[:, 0:dn * D], tp[:, 0:dn * D])

            for r in range(ROUNDS + 1):
                if r < ROUNDS:
                    rows = slice(r * ROWS_PER_ROUND, (r + 1) * ROWS_PER_ROUND)

                    # load in four quarters so folds start as data streams in
                    t = data_pool.tile([P, FREE], F32, tag="t")
                    xr = x[rows, :].rearrange("(p f) d -> p (f d)", p=P)
                    FQ = FREE // 4
                    for qd in range(4):
                        nc.sync.dma_start(
                            t[:, qd * FQ:(qd + 1) * FQ], xr[:, qd * FQ:(qd + 1) * FQ]
                        )

                    # per-partition partials: s cols [0:D]=max, [D:2D]=min.
                    # PSUM ping-pong accumulate: every op reads one SBUF
                    # chunk (dedicated rd0) + the PSUM accumulator -- never
                    # the shared port, so GpSimd runs undisturbed.
                    ppa = ps_pool.tile([P, PPW], F32, tag="ppa")
                    ppb = ps_pool.tile([P, PPW], F32, tag="ppb")
                    s = st1_pool.tile([P, 2 * D], F32, tag="s")
                    for si, op in ((0, OP.max), (1, OP.min)):
                        # init: bypass-copy chunk0 into PSUM (in1 is a dummy
                        # PSUM read, ignored by bypass)
                        nc.vector.tensor_tensor(
                            ppa[:, 0:CH], t[:, 0:CH], ppb[:, 0:CH], op=OP.bypass
                        )
                        cur, nxt = ppa, ppb
                        for c in range(1, NCH):
                            nc.vector.tensor_tensor(
                                nxt[:, 0:CH], cur[:, 0:CH],
                                t[:, c * CH:(c + 1) * CH], op=op,
                            )
                            cur, nxt = nxt, cur
                        # cur holds the [*, 1200] partial; reduce 1200->300.
                        # Two PSUM inputs on one op are illegal, so bounce
                        # half through SBUF with a PSUM-src tensor_scalar
                        # (1x mode, PSUM port only -- no shared-port grab),
                        # then merge with mixed (SBUF, PSUM) operands.
                        sb6 = st1_pool.tile([P, CH // 2], F32, tag="sb6")
                        nc.vector.tensor_scalar_add(
                            sb6[:, 0:CH // 2], cur[:, 0:CH // 2], 0.0
                        )
                        nc.vector.tensor_tensor(
                            nxt[:, 0:CH // 2], sb6[:, 0:CH // 2],
                            cur[:, CH // 2:CH], op=op,
                        )
                        nc.vector.tensor_scalar_add(
                            sb6[:, 0:D], nxt[:, 0:D], 0.0
                        )
                        nc.vector.tensor_tensor(
                            s[:, si * D:(si + 1) * D], sb6[:, 0:D],
                            nxt[:, D:2 * D], op=op,
                        )

                    # gather the Q partials of each graph onto one partition
                    tq = st1_pool.tile([GPR, Q, 2 * D], F32, tag="tq")
                    for q in range(Q):
                        nc.scalar.dma_start(tq[:, q, :], s[q::Q, :])

                if r >= 1:
                    # normalize round r-1, emitted here so the DVE part fills
                    # the gather-DMA latency gap of round r and GpSimd starts
                    # as soon as round r-1's stats landed.
                    emit_normalize(r - 1)

                if r < ROUNDS:
                    # stats tree over the Q pages: every DVE combine is
                    # (SBUF, PSUM) mixed-operand, alternating output spaces.
                    pg = ppa[0:GPR, 0:4 * D].rearrange(
                        "p (a b) -> p a b", a=2, b=2 * D
                    )
                    dummy = ppb[0:GPR, 0:4 * D].rearrange(
                        "p (a b) -> p a b", a=2, b=2 * D
                    )
                    # pages 2,3 -> PSUM (bypass ignores in1)
                    nc.vector.tensor_tensor(
                        pg[:, :, :], tq[:, 2:4, :], dummy[:, :, :], op=OP.bypass
                    )
                    u = st1_pool.tile([GPR, 2 * D], F32, tag="u")
                    pv = ppb[0:GPR, 0:2 * D]
                    w_ps = ppb[0:GPR, 2 * D:4 * D]
                    ssum = st1_pool.tile([GPR, 2 * D], F32, tag="ssum")
                    for si, op in ((0, OP.max), (1, OP.min)):
                        cs = slice(si * D, (si + 1) * D)
                        # L1: (page0 x page2) -> SBUF u, (page1 x page3) -> PSUM pv
                        nc.vector.tensor_tensor(
                            u[:, cs], tq[:, 0, cs], pg[:, 0, cs], op=op
                        )
                        nc.vector.tensor_tensor(
                            pv[:, cs], tq[:, 1, cs], pg[:, 1, cs], op=op
                        )
                        # L2: tmax -> SBUF u2 half, tmin -> PSUM w half
                        dst = u[:, 0:D] if si == 0 else w_ps[:, 0:D]
                        nc.vector.tensor_tensor(dst, u[:, cs], pv[:, cs], op=op)
                    # sum/diff: (SBUF tmax, PSUM tmin) -> SBUF ssum
                    nc.vector.tensor_tensor(
                        ssum[:, 0:D], u[:, 0:D], w_ps[:, 0:D], op=OP.add
                    )
                    nc.vector.tensor_tensor(
                        ssum[:, D:2 * D], u[:, 0:D], w_ps[:, 0:D], op=OP.subtract
                    )

                    # ACT (own ports, never contends): mid = 0.5*sum;
                    # rinv = exp(-ln(relu(0.5*diff - eps) + eps))
                    ab = st2_pool.tile([GPR, 2 * D], F32, tag="ab")
                    nc.scalar.activation(
                        ab[:, 0:D], ssum[:, 0:D], AF.Copy, scale=0.5
                    )
                    nc.scalar.activation(
                        ab[:, D:2 * D], ssum[:, D:2 * D], AF.Relu,
                        bias=-EPS, scale=0.5,
                    )
                    nc.scalar.activation(
                        ab[:, D:2 * D], ab[:, D:2 * D], AF.Ln, bias=EPS
                    )
                    nc.scalar.activation(
                        ab[:, D:2 * D], ab[:, D:2 * D], AF.Exp, scale=-1.0
                    )

                    # replicate stats to all Q partitions of each graph
                    # (SBUF copy for GpSimd), then an ACT copy into PSUM
                    # for DVE's normalize operand.
                    rep = st2_pool.tile([P, 2 * D], F32, tag="rep")
                    for q in range(Q):
                        nc.scalar.dma_start(rep[q::Q, :], ab[:, :])
                    rep_ps = ps_pool.tile([P, 2 * D], F32, tag="rep_ps")
                    nc.scalar.activation(rep_ps[:], rep[:], AF.Copy, scale=1.0)

                    live[r] = (t, rep_ps, rep)

    _split_multi_waits(nc, mybir)
    return nc


def kernel(tensor, batch_list=None, **_ignored):
    """Full-input entry point: tensor [262144, 300] fp32 -> [262144, 300] fp32.

    batch_list is the constant 256-per-graph layout baked into this kernel.
    """
    from concourse.bass_utils import run_bass_kernel_spmd

    tensor = np.ascontiguousarray(np.asarray(tensor), dtype=np.float32)
    assert tensor.shape == (NUM_GRAPHS * NPG, D), tensor.shape

    if "nc" not in _CACHE:
        _CACHE["nc"] = _build()
    nc = _CACHE["nc"]

    in_maps = [
        {"x": tensor[c * ROWS_PER_CORE:(c + 1) * ROWS_PER_CORE]}
        for c in range(N_CORES)
    ]
    res = run_bass_kernel_spmd(nc, in_maps, core_ids=list(range(N_CORES)))
    out = np.concatenate([res.results[c]["y"] for c in range(N_CORES)], axis=0)
    return out


# revision 14
# speedup vs baseline: 1.1380x; 1.1380x over previous
"""Trainium2 Bass kernel for nn_Cube_Norm (segment min/max normalize).

Reference semantics (per graph g of 256 nodes, per dim d):
    tmax = max_n x[g,n,d]; tmin = min_n x[g,n,d]
    mid = (tmax+tmin)/2; ldv = max((tmax-tmin)/2, 1e-12)
    out[g,n,d] = (x[g,n,d] - mid) / ldv

Sharding: 1024 graphs -> 8 cores x 128 graphs (row-sharded at graph
boundaries). Per core, 4 rounds of 32 graphs; each graph occupies 4 SBUF
partitions (64 nodes each): every round is a [128, 19200] fp32 tile with
contiguous DMA in/out (exactly 2x HBM traffic), double-buffered.

Engine split. HW-measured: a DVE op whose two tensor operands are both
in SBUF locks the shared DVE/GpSimd SBUF port pair for the whole
instruction and fully serializes against any GpSimd op (probe: DVE TT
stretched 2.6us -> 15-20us, ending exactly when the GpSimd op ended).
A DVE TT with one operand in PSUM runs at full 1x rate fully overlapped
with GpSimd (probe: 1154ns for 960 elems at 100% GpSimd overlap) --
PSUM has its own DVE port and GpSimd cannot access PSUM. So:
  - Every DVE op here is mixed-operand (<=1 SBUF stream + PSUM stream):
    folds accumulate SBUF chunks into a PSUM ping-pong accumulator;
    the cross-partition stats tree alternates SBUF/PSUM outputs so each
    combine is (SBUF, PSUM); the normalize's mid/rinv broadcast operand
    reads a PSUM replica of the stats.
  - GpSimd concurrently runs the normalize majority slice (stock
    tensor_tensor, ~2.6-3.3 cyc/elem) straight in SBUF.
  - ScalarE (ACT, own ports, never contends) does all stats scaling:
    mid = 0.5*sum, rinv = exp(-ln(relu(0.5*diff - eps) + eps)) (the ACT
    Reciprocal is banned for accuracy; exp/ln error ~1e-4 scales
    multiplicatively with the output so rel-err stays ~1e-4), plus the
    SBUF->PSUM stats replica copy.
  - Loads ride the sync HWDGE ring; stores/stat DMAs ride the scalar
    ring, so stores never head-of-line-block loads.
The last round's normalize (no folds to overlap) rebalances toward DVE.
"""

import numpy as np

NUM_GRAPHS = 1024
NPG = 256            # nodes per graph
D = 300              # embed dim
N_CORES = 8
GPC = NUM_GRAPHS // N_CORES   # 128 graphs per core
ROWS_PER_CORE = GPC * NPG     # 32768
P = 128              # SBUF partitions
Q = 4                # partitions per graph
NPP = NPG // Q       # 64 nodes per partition
GPR = P // Q         # 32 graphs per round
ROUNDS = GPC // GPR  # 4
FREE = NPP * D       # 19200 fp32 per partition per round
ROWS_PER_ROUND = GPR * NPG    # 8192
EPS = 1e-12

# normalize node split: DVE gets [0:dn), GpSimd gets [dn:NPP) in chunks.
DN = 18              # steady-state rounds (folds + tree share DVE)
DN_TAIL = 44         # last round (no folds -> DVE takes the majority)
GN0 = 20             # nodes GpSimd folds in round 0 (otherwise idle there)

CH = 1200            # fold chunk: 4 nodes x 300 dims; PSUM-sized
NCH = FREE // CH     # 16 chunks
PPW = 1536           # ping-pong slot width (3 PSUM banks each)

_CACHE = {}


def _split_multi_waits(nc, mybir, max_waits=1):
    """walrus in this container rejects >N sync waits on one instruction;
    hoist extras into standalone NOPs on the same engine just before."""
    n = 0
    for f in nc.m.functions:
        for bb in f.blocks:
            new_insts = []
            for inst in bb.instructions:
                si = inst.sync_info
                if si is not None and si.on_wait and len(si.on_wait) > max_waits:
                    extra = list(si.on_wait[: len(si.on_wait) - max_waits])
                    keep = list(si.on_wait[len(si.on_wait) - max_waits:])
                    for j, w in enumerate(extra):
                        new_insts.append(
                            mybir.InstNoOp(
                                name=f"{inst.name}-sw{j}",
                                sync_info=mybir.SyncInfo(on_wait=[w], on_update=[]),
                                bass_nofuse=True,
                                engine=inst.engine,
                            )
                        )
                        n += 1
                    inst.sync_info = mybir.SyncInfo(
                        on_wait=keep, on_update=list(si.on_update)
                    )
                new_insts.append(inst)
            bb.instructions.clear()
            for i in new_insts:
                bb.add_instruction(i)
    return n


def _build():
    import concourse.bass as bass
    import concourse.tile as tile
    from concourse import mybir

    F32 = mybir.dt.float32
    OP = mybir.AluOpType
    AF = mybir.ActivationFunctionType

    nc = bass.Bass()
    x = nc.dram_tensor("x", [ROWS_PER_CORE, D], F32, kind="ExternalInput")
    y = nc.dram_tensor("y", [ROWS_PER_CORE, D], F32, kind="ExternalOutput")

    # activation() lowers non-Copy float biases through the const-AP registry,
    # which only pre-registers 0.0/1.0 -- add the eps biases it will need.
    for v in (-EPS, EPS):
        cten = nc.alloc_sbuf_tensor(f"const-f32-eps{'-neg' if v < 0 else ''}",
                                    [128, 1], F32)
        nc.gpsimd.memset(cten.ap(), v)
        nc.const_aps.aps[(F32, v)] = cten.ap()
    nc.all_engine_barrier()

    with tile.TileContext(nc) as tc:
        with tc.tile_pool(name="data", bufs=2) as data_pool, \
             tc.tile_pool(name="st2", bufs=2) as st2_pool, \
             tc.tile_pool(name="st1", bufs=1) as st1_pool, \
             tc.psum_pool(name="ps", bufs=1) as ps_pool:
            live = {}  # r -> (t, rep_ps) awaiting normalize+store

            def emit_normalize(r_prev):
                """Normalize round r_prev in place and store; DVE (PSUM
                stats operand) takes nodes [0:dn), GpSimd the rest in two
                chunks (SBUF stats)."""
                tp, repp_ps, repp_sb = live.pop(r_prev)
                rowsp = slice(r_prev * ROWS_PER_ROUND, (r_prev + 1) * ROWS_PER_ROUND)
                tv3 = tp[:].rearrange("p (n d) -> p n d", n=NPP, d=D)
                yr = y[rowsp, :].rearrange("(p f) d -> p (f d)", p=P)
                dn = DN_TAIL if r_prev == ROUNDS - 1 else DN

                # GpSimd: three chunks, (sub, mul, store) per chunk so the
                # first store fires early and frees the buffer sooner
                gs = NPP - dn
                cuts = [dn, dn + gs // 3, dn + (2 * gs) // 3, NPP]
                for lo, hi in zip(cuts[:-1], cuts[1:]):
                    h = hi - lo
                    mid_b = repp_sb[:, 0:D].unsqueeze(1).broadcast_to([P, h, D])
                    rinv_b = repp_sb[:, D:2 * D].unsqueeze(1).broadcast_to([P, h, D])
                    ns = slice(lo, hi)
                    nc.gpsimd.tensor_tensor(
                        tv3[:, ns, :], tv3[:, ns, :], mid_b, op=OP.subtract
                    )
                    nc.gpsimd.tensor_tensor(
                        tv3[:, ns, :], tv3[:, ns, :], rinv_b, op=OP.mult
                    )
                    nc.scalar.dma_start(
                        yr[:, lo * D:hi * D], tp[:, lo * D:hi * D]
                    )

                # DVE: nodes [0:dn) with the PSUM stats replica as operand,
                # two chunks for earlier partial stores
                dmid = dn // 2
                for lo, hi in ((0, dmid), (dmid, dn)):
                    h = hi - lo
                    mid_b = repp_ps[:, 0:D].unsqueeze(1).broadcast_to([P, h, D])
                    rinv_b = repp_ps[:, D:2 * D].unsqueeze(1).broadcast_to([P, h, D])
                    ns = slice(lo, hi)
                    nc.vector.tensor_tensor(
                        tv3[:, ns, :], tv3[:, ns, :], mid_b, op=OP.subtract
                    )
                    nc.vector.tensor_tensor(
                        tv3[:, ns, :], tv3[:, ns, :], rinv_b, op=OP.mult
                    )
                    nc.scalar.dma_start(yr[:, lo * D:hi * D], tp[:, lo * D:hi * D])

            for r in range(ROUNDS + 1):
                if r < ROUNDS:
                    rows = slice(r * ROWS_PER_ROUND, (r + 1) * ROWS_PER_ROUND)

                    # load in four quarters so folds start as data streams in
                    t = data_pool.tile([P, FREE], F32, tag="t")
                    xr = x[rows, :].rearrange("(p f) d -> p (f d)", p=P)
                    FQ = FREE // 4
                    for qd in range(4):
                        nc.sync.dma_start(
                            t[:, qd * FQ:(qd + 1) * FQ], xr[:, qd * FQ:(qd + 1) * FQ]
                        )

                    # per-partition partials: s cols [0:D]=max, [D:2D]=min.
                    # PSUM ping-pong accumulate: every op reads one SBUF
                    # chunk (dedicated rd0) + the PSUM accumulator -- never
                    # the shared port, so GpSimd runs undisturbed.  Round 0
                    # has no concurrent GpSimd work (its first normalize
                    # needs round 0's stats), so it uses the cheaper plain
                    # SBUF chain (no per-op PSUM access latency).
                    s = st1_pool.tile([P, 2 * D], F32, tag="s")
                    ppa = ps_pool.tile([P, PPW], F32, tag="ppa")
                    ppb = ps_pool.tile([P, PPW], F32, tag="ppb")
                    if r == 0:
                        C0 = 2 * CH
                        for si, op in ((0, OP.max), (1, OP.min)):
                            a = st1_pool.tile([P, C0], F32, tag="fold0")
                            nc.vector.tensor_tensor(
                                a[:], t[:, 0:C0], t[:, C0:2 * C0], op=op
                            )
                            for c in range(2, FREE // C0):
                                nc.vector.tensor_tensor(
                                    a[:], a[:], t[:, c * C0:(c + 1) * C0], op=op
                                )
                            m = C0 // 2
                            while m > D:
                                nc.vector.tensor_tensor(
                                    a[:, 0:m], a[:, 0:m], a[:, m:2 * m], op=op
                                )
                                m //= 2
                            nc.vector.tensor_tensor(
                                s[:, si * D:(si + 1) * D], a[:, 0:D],
                                a[:, D:2 * D], op=op,
                            )
                    else:
                        for si, op in ((0, OP.max), (1, OP.min)):
                            # init: bypass-copy chunk0 into PSUM (in1 is a
                            # dummy PSUM read, ignored by bypass)
                            nc.vector.tensor_tensor(
                                ppa[:, 0:CH], t[:, 0:CH], ppb[:, 0:CH],
                                op=OP.bypass,
                            )
                            cur, nxt = ppa, ppb
                            for c in range(1, NCH):
                                nc.vector.tensor_tensor(
                                    nxt[:, 0:CH], cur[:, 0:CH],
                                    t[:, c * CH:(c + 1) * CH], op=op,
                                )
                                cur, nxt = nxt, cur
                            # cur holds the [*, 1200] partial; reduce to 300.
                            # Two PSUM inputs on one op are illegal, so
                            # bounce half through SBUF with a PSUM-src
                            # tensor_scalar (1x mode, PSUM port only -- no
                            # shared-port grab), then merge mixed-operand.
                            sb6 = st1_pool.tile([P, CH // 2], F32, tag="sb6")
                            nc.vector.tensor_scalar_add(
                                sb6[:, 0:CH // 2], cur[:, 0:CH // 2], 0.0
                            )
                            nc.vector.tensor_tensor(
                                nxt[:, 0:CH // 2], sb6[:, 0:CH // 2],
                                cur[:, CH // 2:CH], op=op,
                            )
                            nc.vector.tensor_scalar_add(
                                sb6[:, 0:D], nxt[:, 0:D], 0.0
                            )
                            nc.vector.tensor_tensor(
                                s[:, si * D:(si + 1) * D], sb6[:, 0:D],
                                nxt[:, D:2 * D], op=op,
                            )

                    # gather the Q partials of each graph onto one partition
                    # (sync ring: the scalar ring's stores would head-of-line
                    # block these latency-critical little DMAs)
                    tq = st1_pool.tile([GPR, Q, 2 * D], F32, tag="tq")
                    for q in range(Q):
                        nc.sync.dma_start(tq[:, q, :], s[q::Q, :])

                if r >= 1:
                    # normalize round r-1, emitted here so the DVE part fills
                    # the gather-DMA latency gap of round r and GpSimd starts
                    # as soon as round r-1's stats landed.
                    emit_normalize(r - 1)

                if r < ROUNDS:
                    # stats tree over the Q pages: every DVE combine is
                    # (SBUF, PSUM) mixed-operand, alternating output spaces.
                    pg = ppa[0:GPR, 0:4 * D].rearrange(
                        "p (a b) -> p a b", a=2, b=2 * D
                    )
                    dummy = ppb[0:GPR, 0:4 * D].rearrange(
                        "p (a b) -> p a b", a=2, b=2 * D
                    )
                    # pages 2,3 -> PSUM (bypass ignores in1)
                    nc.vector.tensor_tensor(
                        pg[:, :, :], tq[:, 2:4, :], dummy[:, :, :], op=OP.bypass
                    )
                    u = st1_pool.tile([GPR, 2 * D], F32, tag="u")
                    pv = ppb[0:GPR, 0:2 * D]
                    w_ps = ppb[0:GPR, 2 * D:4 * D]
                    ssum = st1_pool.tile([GPR, 2 * D], F32, tag="ssum")
                    for si, op in ((0, OP.max), (1, OP.min)):
                        cs = slice(si * D, (si + 1) * D)
                        # L1: (page0 x page2) -> SBUF u, (page1 x page3) -> PSUM pv
                        nc.vector.tensor_tensor(
                            u[:, cs], tq[:, 0, cs], pg[:, 0, cs], op=op
                        )
                        nc.vector.tensor_tensor(
                            pv[:, cs], tq[:, 1, cs], pg[:, 1, cs], op=op
                        )
                        # L2: tmax -> SBUF u2 half, tmin -> PSUM w half
                        dst = u[:, 0:D] if si == 0 else w_ps[:, 0:D]
                        nc.vector.tensor_tensor(dst, u[:, cs], pv[:, cs], op=op)
                    # sum/diff: (SBUF tmax, PSUM tmin) -> SBUF ssum
                    nc.vector.tensor_tensor(
                        ssum[:, 0:D], u[:, 0:D], w_ps[:, 0:D], op=OP.add
                    )
                    nc.vector.tensor_tensor(
                        ssum[:, D:2 * D], u[:, 0:D], w_ps[:, 0:D], op=OP.subtract
                    )

                    # ACT (own ports, never contends): mid = 0.5*sum;
                    # rinv = exp(-ln(relu(0.5*diff - eps) + eps))
                    ab = st2_pool.tile([GPR, 2 * D], F32, tag="ab")
                    nc.scalar.activation(
                        ab[:, 0:D], ssum[:, 0:D], AF.Copy, scale=0.5
                    )
                    nc.scalar.activation(
                        ab[:, D:2 * D], ssum[:, D:2 * D], AF.Relu,
                        bias=-EPS, scale=0.5,
                    )
                    nc.scalar.activation(
                        ab[:, D:2 * D], ab[:, D:2 * D], AF.Ln, bias=EPS
                    )
                    nc.scalar.activation(
                        ab[:, D:2 * D], ab[:, D:2 * D], AF.Exp, scale=-1.0
                    )

                    # replicate stats to all Q partitions of each graph
                    # (SBUF copy for GpSimd), then an ACT copy into PSUM
                    # for DVE's normalize operand.
                    rep = st2_pool.tile([P, 2 * D], F32, tag="rep")
                    for q in range(Q):
                        nc.sync.dma_start(rep[q::Q, :], ab[:, :])
                    rep_ps = ps_pool.tile([P, 2 * D], F32, tag="rep_ps")
                    nc.scalar.activation(rep_ps[:], rep[:], AF.Copy, scale=1.0)

                    live[r] = (t, rep_ps, rep)

    _split_multi_waits(nc, mybir)
    return nc


def kernel(tensor, batch_list=None, **_ignored):
    """Full-input entry point: tensor [262144, 300] fp32 -> [262144, 300] fp32.

    batch_list is the constant 256-per-graph layout baked into this kernel.
    """
    from concourse.bass_utils import run_bass_kernel_spmd

    tensor = np.ascontiguousarray(np.asarray(tensor), dtype=np.float32)
    assert tensor.shape == (NUM_GRAPHS * NPG, D), tensor.shape

    if "nc" not in _CACHE:
        _CACHE["nc"] = _build()
    nc = _CACHE["nc"]

    in_maps = [
        {"x": tensor[c * ROWS_PER_CORE:(c + 1) * ROWS_PER_CORE]}
        for c in range(N_CORES)
    ]
    res = run_bass_kernel_spmd(nc, in_maps, core_ids=list(range(N_CORES)))
    out = np.concatenate([res.results[c]["y"] for c in range(N_CORES)], axis=0)
    return out
